# revision 24
# baseline (speedup 1.0000x reference)
"""Trainium2 Bass kernel for HPEncoder sparse-conv network (v2).

Network (C=128, f32 in/out):
  h = relu(conv0(x))   27-offset stride-1 sparse conv, N0=200000 voxels
  h = conv1(h)         27-offset stride-1
  h = relu(down1(h))   8-offset stride-2 -> N1 voxels
  h = conv2(h)         27-offset stride-1 at level 1
  out = down2(h)       8-offset stride-2 -> N2=8000 voxels

Distribution: level-2 output grid (20^3, fully occupied) is split into 8
octants; each core's working set for earlier layers is the backward closure
of its octant (ghost zones, no inter-core communication).

Per conv, per 512-row output tile: ONE SWDGE dma_gather(transpose=True)
fetches all K*512 bf16 input rows channels-major (the exact matmul rhs
layout), K matmuls accumulate W_k^T rows into a PSUM bank, bias(+relu) is
fused into the scalar-engine eviction, a PE transpose restores row-major
layout, and HWDGE stores write the bf16 feature table for the next layer.

dma_gather indices are int16, so each tile gathers through a 32768-row
window of the source table (host-chosen base per tile, uniform across
cores); tables carry a zero row every ZP=8192 positions so every window
contains one (absent neighbors gather zeros).
"""

import itertools
import numpy as np
import ml_dtypes

P = 128
C = 128
TS = 512           # output rows per tile (one PSUM bank)
Q = TS // P
ZP = 8192          # zero-row period (table positions)
ZB = ZP - 1        # data rows per zero-row block
WMAX = 32768       # int16 gather window size
GK = 1             # offsets per dma_gather instruction; at NI=512 the
                   # single_packet descgen path is legal (crashes >=1024) --
                   # probing whether packed generation beats the per-packet
                   # path (GK=8 90.5ms, GK=4 89.95ms, whole-tile 108ms)


def _groups(K):
    return [(k0, min(k0 + GK, K)) for k0 in range(0, K, GK)]

_cache = {}
TRACE = False
TRACE_CORES = None


def _pos(d):
    """Data row index -> table position (zero rows at multiples of ZP)."""
    return d + d // ZB + 1


def _rp(rd):
    """Table positions needed for rd data rows."""
    return int(_pos(rd - 1)) + 1


def _zrows(rd):
    """Zero-row positions for a table with rd data rows."""
    return list(range(0, _rp(rd), ZP))


def _sorted_map(im, om):
    im = np.asarray(im).copy()
    om = np.asarray(om).copy()
    for k in range(im.shape[0]):
        o = np.argsort(om[k], kind="stable")
        im[k], om[k] = im[k][o], om[k][o]
    return im, om


def _closure(need_out, im, om):
    """Input-table rows needed to produce output rows `need_out` (sorted)."""
    K, n = im.shape
    need = [np.empty(0, np.int64)]
    for k in range(K):
        omk, imk = om[k], im[k]
        pos = np.searchsorted(omk, need_out)
        pos = np.minimum(pos, n - 1)
        # pad entries carry an out-row sentinel that never matches a real id,
        # so `hit` alone excludes them (the im pad sentinel differs per map
        # kind: n for stride-1, n_in for down maps)
        hit = omk[pos] == need_out
        need.append(imk[pos][hit].astype(np.int64))
    return np.unique(np.concatenate(need))


def _pad_rows(rows, rd):
    out = np.full(rd, -1, np.int64)
    out[:len(rows)] = rows
    return out


def _tile_bases(T, rp):
    """Per-tile gather window (base, size, zero-row position)."""
    bases, wins, zps = [], [], []
    for t in range(T):
        base = min(max(int(_pos(t * TS)) - ZP, 0), max(0, rp - WMAX))
        w = min(WMAX, rp - base)
        z = -(-base // ZP) * ZP
        assert base <= z < base + w
        bases.append(base)
        wins.append(w)
        zps.append(z)
    return bases, wins, zps


def _store_segs(T, plain):
    """Per-tile store segments [(q, part_off, nrows, dram_row), ...]."""
    segs = []
    for t in range(T):
        s = []
        for q in range(Q):
            d0 = t * TS + q * P
            if plain:
                s.append((q, 0, P, d0))
            elif d0 // ZB == (d0 + P - 1) // ZB:
                s.append((q, 0, P, int(_pos(d0))))
            else:
                ds = (d0 // ZB + 1) * ZB
                s.append((q, 0, ds - d0, int(_pos(d0))))
                s.append((q, ds - d0, P - (ds - d0), int(_pos(ds))))
        segs.append(s)
    return segs


def _build_ix(out_rows, im, om, in_ids, in_kp, bases, wins, zps):
    """[T, 128, K*TS/16] int16 gather-index tensor for one core+layer.

    Gather element i = k*TS + s feeds rhs column (k, s): the window-relative
    position of the input row for output out_rows[t*TS+s] at offset k, or of
    the tile's zero row when absent. Wrapped [i%16, i//16], replicated 8x
    across partition groups (one stripe per GPSIMD Q7 core).

    out_rows: key-ordered global out ids, -1 padded. in_ids: id-sorted global
    input ids; in_kp maps id-rank -> key-position in the input table.
    """
    K, n = im.shape
    rd = len(out_rows)
    T = rd // TS
    NI = K * TS
    ncols = NI // 16
    loc = np.full((K, rd), -1, np.int64)
    valid = out_rows >= 0
    ov = out_rows[valid]
    for k in range(K):
        omk, imk = om[k], im[k]
        pos = np.searchsorted(omk, ov)
        pos = np.minimum(pos, n - 1)
        hit = omk[pos] == ov
        src = imk[pos]
        l = np.searchsorted(in_ids, src)
        l = np.minimum(l, max(len(in_ids) - 1, 0))
        ok = hit & (in_ids[l] == src)
        col = np.full(len(ov), -1, np.int64)
        col[ok] = in_kp[l[ok]]
        loc[k, valid] = col
    pp = np.where(loc >= 0, _pos(loc), -1)
    ix = np.empty((T, 128, ncols), np.int16)
    for t in range(T):
        sl = pp[:, t * TS:(t + 1) * TS]
        rel = np.where(sl >= 0, sl - bases[t], zps[t] - bases[t])
        assert rel.min() >= 0 and rel.max() < wins[t], \
            (t, rel.min(), rel.max(), wins[t])
        # wrap each gather group independently: group columns hold its own
        # (i%16, i//16) layout so per-group SWDGE calls can slice the tile
        for (k0, k1) in _groups(K):
            nig = (k1 - k0) * TS
            w = rel[k0:k1].reshape(nig).astype(np.int16) \
                .reshape(nig // 16, 16).T
            ix[t, :, k0 * (TS // 16):k1 * (TS // 16)] = np.tile(w, (8, 1))
    return ix


def _recover_coords(din, dout, out_xyz, n_in):
    """L(v) coords from the Lv->Lv+1 down map and Lv+1 coords. Each input
    row appears in exactly one parity class: coords = out*2 + offset."""
    xyz = np.full((n_in, 3), -1, np.int64)
    offs = list(itertools.product((0, 1), repeat=3))
    for k, off in enumerate(offs):
        v = din[k] < n_in
        xyz[din[k][v]] = out_xyz[dout[k][v]] * 2 + np.asarray(off)
    assert (xyz >= 0).all()
    return xyz


def _key_table(ids, key):
    """Key-ordered local table + (id-rank -> key-position) lookup.
    `ids` is unique-sorted; returns (key-ordered ids, keypos)."""
    order = np.argsort(key[ids], kind="stable")
    keypos = np.empty(len(ids), np.int64)
    keypos[order] = np.arange(len(ids))
    return ids[order], keypos


def _plan(inputs):
    in0, out0 = _sorted_map(inputs["in0"], inputs["out0"])
    in1, out1 = _sorted_map(inputs["in1"], inputs["out1"])
    din1, dout1 = _sorted_map(inputs["din1"], inputs["dout1"])
    din2, dout2 = _sorted_map(inputs["din2"], inputs["dout2"])
    N0 = in0.shape[1]
    N1 = din1.shape[1]
    N2 = din2.shape[1]

    # level-0 row ids are randomly ordered; recover spatial keys through the
    # down maps (level-2 is the dense sorted 20^3 grid) and key-sort every
    # local table so gather windows are spatially local.
    assert N2 == 8000, "assumes dense 20^3 level-2 grid"
    xyz2 = np.stack(np.unravel_index(np.arange(N2), (20, 20, 20)), axis=1)
    xyz1 = _recover_coords(np.asarray(inputs["din2"]),
                           np.asarray(inputs["dout2"]), xyz2, N1)
    xyz0 = _recover_coords(np.asarray(inputs["din1"]),
                           np.asarray(inputs["dout1"]), xyz1, N0)
    key0 = (xyz0[:, 0] * 80 + xyz0[:, 1]) * 80 + xyz0[:, 2]
    key1 = (xyz1[:, 0] * 40 + xyz1[:, 1]) * 40 + xyz1[:, 2]

    cores = []
    for cx, cy, cz in itertools.product((0, 1), repeat=3):
        m = ((xyz2[:, 0] >= 10) == bool(cx)) & \
            ((xyz2[:, 1] >= 10) == bool(cy)) & \
            ((xyz2[:, 2] >= 10) == bool(cz))
        s2 = np.nonzero(m)[0].astype(np.int64)
        a1 = _closure(s2, din2, dout2)
        b1 = _closure(a1, in1, out1)
        a0 = _closure(b1, din1, dout1)
        b0 = _closure(a0, in0, out0)
        c0 = _closure(b0, in0, out0)
        cc = dict(s2=s2, a1=a1, b1=b1, a0=a0, b0=b0, c0=c0)
        # key-ordered tables (kt_*) + id-rank -> key-position lookups (kp_*)
        for nm, key in (("c0", key0), ("b0", key0), ("a0", key0),
                        ("b1", key1), ("a1", key1)):
            cc["kt_" + nm], cc["kp_" + nm] = _key_table(cc[nm], key)
        cc["kt_s2"], cc["kp_s2"] = s2, np.arange(len(s2))
        cores.append(cc)

    def rd_of(key, mult=TS):
        return -(-max(len(cc[key]) for cc in cores) // mult) * mult

    plan = dict(cores=cores, N2=N2,
                rd_x=rd_of("c0", 1), rd_b0=rd_of("b0"), rd_a0=rd_of("a0"),
                rd_b1=rd_of("b1"), rd_a1=rd_of("a1"), rd_s2=rd_of("s2"))
    plan["rp_x"] = _rp(plan["rd_x"])
    plan["rp_b0"] = _rp(plan["rd_b0"])
    plan["rp_a0"] = _rp(plan["rd_a0"])
    plan["rp_b1"] = _rp(plan["rd_b1"])
    plan["rp_a1"] = _rp(plan["rd_a1"])

    # uniform per-layer tile geometry
    geom = {}
    for nm, rd, rp_in, K in (("c0", plan["rd_b0"], plan["rp_x"], 27),
                             ("c1", plan["rd_a0"], plan["rp_b0"], 27),
                             ("d1", plan["rd_b1"], plan["rp_a0"], 8),
                             ("c2", plan["rd_a1"], plan["rp_b1"], 27),
                             ("d2", plan["rd_s2"], plan["rp_a1"], 8)):
        T = rd // TS
        bases, wins, zps = _tile_bases(T, rp_in)
        geom[nm] = dict(T=T, K=K, bases=bases, wins=wins, zps=zps,
                        segs=_store_segs(T, plain=(nm == "d2")))
    plan["geom"] = geom

    for cc in cores:
        b0p = _pad_rows(cc["kt_b0"], plan["rd_b0"])
        a0p = _pad_rows(cc["kt_a0"], plan["rd_a0"])
        b1p = _pad_rows(cc["kt_b1"], plan["rd_b1"])
        a1p = _pad_rows(cc["kt_a1"], plan["rd_a1"])
        s2p = _pad_rows(cc["kt_s2"], plan["rd_s2"])
        g = geom
        cc["ix_c0"] = _build_ix(b0p, in0, out0, cc["c0"], cc["kp_c0"],
                                g["c0"]["bases"], g["c0"]["wins"],
                                g["c0"]["zps"])
        cc["ix_c1"] = _build_ix(a0p, in0, out0, cc["b0"], cc["kp_b0"],
                                g["c1"]["bases"], g["c1"]["wins"],
                                g["c1"]["zps"])
        cc["ix_d1"] = _build_ix(b1p, din1, dout1, cc["a0"], cc["kp_a0"],
                                g["d1"]["bases"], g["d1"]["wins"],
                                g["d1"]["zps"])
        cc["ix_c2"] = _build_ix(a1p, in1, out1, cc["b1"], cc["kp_b1"],
                                g["c2"]["bases"], g["c2"]["wins"],
                                g["c2"]["zps"])
        cc["ix_d2"] = _build_ix(s2p, din2, dout2, cc["a1"], cc["kp_a1"],
                                g["d2"]["bases"], g["d2"]["wins"],
                                g["d2"]["zps"])
    return plan


def _build_module(plan):
    import concourse.bass as bass
    import concourse.bacc as bacc
    import concourse.mybir as mybir
    import concourse.tile as tile
    from concourse.masks import make_identity

    F32 = mybir.dt.float32
    BF16 = mybir.dt.bfloat16
    I16 = mybir.dt.int16
    nc = bacc.Bacc("TRN2", target_bir_lowering=False, debug=False,
                   num_devices=8)

    g = plan["geom"]
    xt = nc.dram_tensor("xt", [plan["rp_x"], C], BF16, kind="ExternalInput").ap()
    tb0 = nc.dram_tensor("tb0", [plan["rp_b0"], C], BF16, kind="Internal").ap()
    ta0 = nc.dram_tensor("ta0", [plan["rp_a0"], C], BF16, kind="Internal").ap()
    tb1 = nc.dram_tensor("tb1", [plan["rp_b1"], C], BF16, kind="Internal").ap()
    ta1 = nc.dram_tensor("ta1", [plan["rp_a1"], C], BF16, kind="Internal").ap()
    out = nc.dram_tensor("out", [plan["rd_s2"], C], F32, kind="ExternalOutput").ap()

    ws, bs, ixs = {}, {}, {}
    for nm, K in (("W0", 27), ("W1", 27), ("Wd1", 8), ("W2", 27), ("Wd2", 8)):
        ws[nm] = nc.dram_tensor(nm, [K, C, C], BF16, kind="ExternalInput").ap()
    for nm in ("b0", "b1", "bd1", "b2", "bd2"):
        bs[nm] = nc.dram_tensor(nm, [C, 1], F32, kind="ExternalInput").ap()
    for nm, gk in (("ix_c0", "c0"), ("ix_c1", "c1"), ("ix_d1", "d1"),
                   ("ix_c2", "c2"), ("ix_d2", "d2")):
        T, K = g[gk]["T"], g[gk]["K"]
        ixs[nm] = nc.dram_tensor(nm, [T, 128, K * TS // 16], I16,
                                 kind="ExternalInput").ap()

    with tile.TileContext(nc) as tc:
        with tc.tile_pool(name="wp", bufs=1) as wp, \
             tc.tile_pool(name="gp", bufs=2) as gp, \
             tc.tile_pool(name="ixp", bufs=3) as ixp, \
             tc.tile_pool(name="ev", bufs=3) as ev, \
             tc.tile_pool(name="pso", bufs=3, space="PSUM") as pso, \
             tc.tile_pool(name="ps", bufs=2, space="PSUM") as ps:

            identb = wp.tile([P, P], BF16)
            make_identity(nc, identb[:])
            identf = wp.tile([P, P], F32)
            make_identity(nc, identf[:])

            # zero rows of internal tables
            zt = wp.tile([1, C], BF16)
            nc.vector.memset(zt[:], 0.0)
            for tab, rd in ((tb0, plan["rd_b0"]), (ta0, plan["rd_a0"]),
                            (tb1, plan["rd_b1"]), (ta1, plan["rd_a1"])):
                for zp in _zrows(rd):
                    nc.sync.dma_start(out=tab[zp:zp + 1, :], in_=zt[:1, :])

            def conv(gk, ftab_in, ftab_out, ix_ap, Wap, bap, relu, last):
                gg = g[gk]
                T, K = gg["T"], gg["K"]
                NI = K * TS
                wt = wp.tile([P, K * C], BF16, tag=f"w_{gk}")
                for k in range(K):
                    nc.sync.dma_start(out=wt[:, k * C:(k + 1) * C],
                                      in_=Wap[k, :, :])
                bt = wp.tile([P, 1], F32, tag=f"b_{gk}")
                nc.sync.dma_start(out=bt[:], in_=bap[:, :])
                act = (mybir.ActivationFunctionType.Relu if relu
                       else mybir.ActivationFunctionType.Identity)
                odt = F32 if last else BF16
                ident = identf if last else identb
                for t in range(T):
                    base, win = gg["bases"][t], gg["wins"][t]
                    ixt = ixp.tile([128, NI // 16], I16, tag="ixt")
                    nc.sync.dma_start(out=ixt[:], in_=ix_ap[t, :, :])
                    gts = []
                    for gi, (k0, k1) in enumerate(_groups(K)):
                        nig = (k1 - k0) * TS
                        gt = gp.tile([128, 1, nig], BF16, tag=f"g{gi}")
                        nc.gpsimd.dma_gather(
                            out_ap=gt[:, :, :],
                            in_ap=ftab_in[base:base + win, :],
                            idxs_ap=ixt[:, k0 * (TS // 16):k1 * (TS // 16)],
                            num_idxs=nig, num_idxs_reg=nig,
                            elem_size=C, transpose=True,
                            single_packet=(nig <= 768))
                        gts.append(gt)
                    po = pso.tile([P, TS], F32, space="PSUM", tag="po")
                    for k in range(K):
                        gi, k0 = k // GK, (k // GK) * GK
                        nc.tensor.matmul(out=po[:],
                                         lhsT=wt[:, k * C:(k + 1) * C],
                                         rhs=gts[gi][:, 0,
                                                     (k - k0) * TS:
                                                     (k - k0 + 1) * TS],
                                         start=(k == 0), stop=(k == K - 1))
                    ot = ev.tile([P, TS], odt, tag="ot")
                    nc.scalar.activation(out=ot[:], in_=po[:], func=act,
                                         bias=bt[:])
                    tp = ps.tile([P, TS], odt, space="PSUM", tag="tp")
                    for q in range(Q):
                        nc.tensor.transpose(out=tp[:, q * P:(q + 1) * P],
                                            in_=ot[:, q * P:(q + 1) * P],
                                            identity=ident[:])
                    orow = ev.tile([P, TS], odt, tag="orow")
                    nc.vector.tensor_copy(out=orow[:], in_=tp[:])
                    for (q, off, n, dr) in gg["segs"][t]:
                        nc.sync.dma_start(
                            out=ftab_out[dr:dr + n, :],
                            in_=orow[off:off + n, q * P:(q + 1) * P])

            conv("c0", xt, tb0, ixs["ix_c0"], ws["W0"], bs["b0"], True, False)
            conv("c1", tb0, ta0, ixs["ix_c1"], ws["W1"], bs["b1"], False, False)
            conv("d1", ta0, tb1, ixs["ix_d1"], ws["Wd1"], bs["bd1"], True, False)
            conv("c2", tb1, ta1, ixs["ix_c2"], ws["W2"], bs["b2"], False, False)
            conv("d2", ta1, out, ixs["ix_d2"], ws["Wd2"], bs["bd2"], False, True)
    nc.compile()
    return nc


def kernel(**inputs):
    if "plan" not in _cache:
        _cache["plan"] = _plan(inputs)
    plan = _cache["plan"]
    if "nc" not in _cache:
        _cache["nc"] = _build_module(plan)
    nc = _cache["nc"]

    x = np.asarray(inputs["x"], np.float32)

    def wmat(nm):
        return np.ascontiguousarray(
            np.asarray(inputs[nm], np.float32)).astype(ml_dtypes.bfloat16)

    def bvec(nm):
        return np.ascontiguousarray(
            np.asarray(inputs[nm], np.float32).reshape(C, 1))

    shared = dict(W0=wmat("W0"), W1=wmat("W1"), Wd1=wmat("Wd1"),
                  W2=wmat("W2"), Wd2=wmat("Wd2"),
                  b0=bvec("b0"), b1=bvec("b1"), bd1=bvec("bd1"),
                  b2=bvec("b2"), bd2=bvec("bd2"))

    in_maps = []
    for cc in plan["cores"]:
        xt = np.zeros((plan["rp_x"], C), ml_dtypes.bfloat16)
        n = len(cc["c0"])
        xt[_pos(np.arange(n))] = x[cc["kt_c0"]].astype(ml_dtypes.bfloat16)
        m = dict(xt=xt, **shared,
                 ix_c0=cc["ix_c0"], ix_c1=cc["ix_c1"], ix_d1=cc["ix_d1"],
                 ix_c2=cc["ix_c2"], ix_d2=cc["ix_d2"])
        in_maps.append(m)

    from concourse.bass_utils import run_bass_kernel_spmd
    res = run_bass_kernel_spmd(nc, in_maps, core_ids=list(range(8)),
                               trace=TRACE, trace_cores=TRACE_CORES)
    _cache["last"] = res

    out_full = np.zeros((plan["N2"], C), np.float32)
    for c, cc in enumerate(plan["cores"]):
        s2 = cc["s2"]
        out_full[s2] = res.results[c]["out"][:len(s2)]
    _cache["in_maps"] = in_maps
    return out_full


def bench(iters=12):
    """Re-run the compiled module with device-resident inputs; return the
    per-execution wall times (s). Call kernel(...) first."""
    import time
    import jax
    import jax.numpy as jnp
    from jax.sharding import Mesh, PartitionSpec, NamedSharding
    from jax.experimental.shard_map import shard_map
    import concourse.mybir as mybir
    from concourse import bass2jax as b2j

    nc = _cache["nc"]
    in_maps = _cache["in_maps"]
    b2j.install_neuronx_cc_hook()
    n_cores = len(in_maps)

    partition_name = (nc.partition_id_tensor.name
                      if nc.partition_id_tensor else None)
    in_names, out_names, out_avals, zero_outs = [], [], [], []
    for alloc in nc.m.functions[0].allocations:
        if not isinstance(alloc, mybir.MemoryLocationSet):
            continue
        name = alloc.memorylocations[0].name
        if alloc.kind == "ExternalInput":
            if name != partition_name:
                in_names.append(name)
        elif alloc.kind == "ExternalOutput":
            out_names.append(name)
            shape = tuple(alloc.tensor_shape)
            dtype = mybir.dt.np(alloc.dtype)
            out_avals.append(jax.core.ShapedArray(shape, dtype))
            zero_outs.append(np.zeros(shape, dtype))
    n_params = len(in_names)
    all_in = in_names + out_names + ([partition_name] if partition_name else [])

    def _body(*args):
        operands = list(args)
        if partition_name is not None:
            operands.append(b2j.partition_id_tensor())
        return tuple(b2j._bass_exec_p.bind(
            *operands, out_avals=tuple(out_avals), in_names=tuple(all_in),
            out_names=tuple(out_names), lowering_input_output_aliases=(),
            sim_require_finite=True, sim_require_nnan=True, nc=nc))

    devices = jax.devices()[:n_cores]
    mesh = Mesh(np.asarray(devices), ("core",))
    nin = n_params + len(out_names)
    fn = jax.jit(shard_map(_body, mesh=mesh,
                           in_specs=(PartitionSpec("core"),) * nin,
                           out_specs=(PartitionSpec("core"),) * len(out_names),
                           check_rep=False))
    sh = NamedSharding(mesh, PartitionSpec("core"))
    args = []
    for i, name in enumerate(in_names):
        cat = np.concatenate([np.asarray(m[name]) for m in in_maps], axis=0)
        args.append(jax.device_put(cat, sh))
    for z in zero_outs:
        cat = np.zeros((n_cores * z.shape[0], *z.shape[1:]), z.dtype)
        args.append(jax.device_put(cat, sh))
    # warmup (compile + first exec)
    out = fn(*args)
    jax.block_until_ready(out)
    walls = []
    for _ in range(iters):
        t0 = time.time()
        out = fn(*args)
        jax.block_until_ready(out)
        walls.append(time.time() - t0)
    return walls
